# revision 79
# baseline (speedup 1.0000x reference)
"""Masked video loss kernel for TRN2 (8 NeuronCores, SPMD).

Algorithmic structure exploited:
- The decoder input feat_3d is spatially constant (broadcast of per-frame
  features over H=W=64), so conv1 collapses to a per-frame linear map with
  9 edge-variant weight sums (W1eff), evaluated directly on a 5x5 class
  grid. conv2 runs as a true 3x3x3 conv on the (padded) 5-grid; its output
  is expanded to the 7-grid along the h-axis only (conv3's stationary
  slices address the w-axis 5-grid directly via a b-row lookup) and conv3
  produces the 7x7 recon classes. All exact (class algebra), not
  approximations.
- Masked MSE folds through per-class stats: sum (r-o)^2 = r^2 cnt - 2 r s1
  + s2 per (t, 7x7 class); s1/cnt come from one 0/1-matrix PE matmul plus
  segmented DVE reduces.

Sharding: core = 2*b + th. Each core handles batch b and an 11-frame
t-window starting at s = 5*th (host shifts the data, so the program is
SPMD-uniform); decoder outputs are valid for the core's 8-frame t-half,
and mask stats are host-zeroed outside that half.

Precision: W_enc / w1eff / w2 / obs in fp8-e4m3, masks u8, activations
bf16, accumulation fp32. Measured end-to-end loss rel err 3.2e-3
(harness gate 2e-2).
"""

import sys

sys.path.insert(0, "/opt/trn_rl_repo")

from contextlib import ExitStack  # noqa: E402

import numpy as np  # noqa: E402

import concourse.bacc as bacc  # noqa: E402
import concourse.mybir as mybir  # noqa: E402
import concourse.tile as tile  # noqa: E402
from concourse import bass_utils  # noqa: E402

B, T, C, H, W = 4, 16, 3, 64, 64
D = 256
NCORES = 8

F32 = mybir.dt.float32
BF16 = mybir.dt.bfloat16
F8 = mybir.dt.float8e4

WIN = 11          # feats/conv t-window frames per core
WP = WIN + 2      # padded window
M35 = [0, 1, 1, 1, 2]          # 5-grid pos -> 3-class variant
M57 = [0, 1, 2, 2, 2, 3, 4]    # 7-grid pos -> 5-grid src index
# expansion groups (dst0, dstlen, src0, srclen) along one axis for 5->7
G57 = [(0, 2, 0, 2), (2, 3, 2, 1), (5, 2, 3, 2)]
# h2-row groups for segmented stats reduction (h = 2*h2 + hpar)
H2G = [(0, 1), (1, 2), (2, 30), (30, 31), (31, 32)]
NG = len(H2G)
WCLS_BOUNDS = [0, 1, 2, 3, 61, 62, 63, 64]

NU1 = 3 * NG * WIN       # 165
NUC = NG * WIN           # 55
NSTAT = NU1 + NUC + 1    # 221


def _emit(nc, a_in, a_out):
    ctx = ExitStack()
    tc = tile.TileContext(nc)
    with tc, ctx:
        io = ctx.enter_context(tc.tile_pool(name="io", bufs=1))
        wkp = ctx.enter_context(tc.tile_pool(name="wkp", bufs=3))
        work = ctx.enter_context(tc.tile_pool(name="work", bufs=1))
        ps = ctx.enter_context(tc.tile_pool(name="ps", bufs=1, space="PSUM"))

        # ---------- early memsets (Pool; no deps) ----------
        fpad = work.tile([128, 2 * WP], BF16, tag="fpad")
        nc.gpsimd.memset(fpad[:], 0.0)
        h1p = work.tile([128, WP * 7 * 7], BF16, tag="h1p")
        nc.gpsimd.memset(h1p[:], 0.0)
        h2p = work.tile([65, 7 * WP * 9 + 2], BF16, tag="h2p")
        nc.gpsimd.memset(h2p[0:64, :], 0.0)
        nc.gpsimd.memset(h2p[64:65, :], 1.0)

        # ---------- input DMAs (serialized by the DMA engine) ----------
        # one merged leading tensor: [bf16 consts block | fp8 obs | u8 masks]
        in0 = io.tile([128, 2176], mybir.dt.uint8)
        nc.sync.dma_start(in0[:], a_in["in0"])
        om = in0[:, 0:416].bitcast(BF16)   # [128, 208]
        obs8 = in0[:, 416:1472].bitcast(F8)
        msk8 = in0[:, 1472:2176]
        obsb = work.tile([128, 1056], BF16, tag="obsb")
        obs = obsb[:, 0:1056]
        kmbf = work.tile([128, 704], BF16, tag="kmbf")
        keep = kmbf[:, 0:352]
        mst = kmbf[:, 352:704]
        nc.vector.tensor_copy(keep, msk8[:, 0:352])  # u8 -> bf16

        # ---------- PE warm-up (p-state ramp) ----------
        warm_ps = ps.tile([2, 512], F32, tag="warm")
        for i in range(24):
            nc.tensor.matmul(
                warm_ps[:, 0:8], om[:, 0:2], om[:, 0:8],
                start=(i == 0), stop=(i == 23),
            )
        # early act-table preload (off the critical path)
        junk = work.tile([2, 8], F32, tag="junk")
        nc.scalar.activation(
            junk[:], warm_ps[:, 0:8], mybir.ActivationFunctionType.Relu
        )
        for i in range(10):
            nc.tensor.matmul(
                warm_ps[:, 0:128], om[:, 0:2], om[:, 0:128],
                start=(i == 0), stop=(i == 9),
            )

        # ---------- encoder input masking (cast + mask per c-block) ----
        xt = work.tile([128, 3 * 32 * WIN], BF16, tag="xt")
        for c in range(C):
            sl = slice(c * 32 * WIN, (c + 1) * 32 * WIN)
            nc.vector.tensor_copy(obsb[:, sl], obs8[:, sl])
            nc.vector.tensor_mul(xt[:, sl], obs[:, sl], keep)
        nc.vector.tensor_copy(mst, msk8[:, 352:704])

        # bias vectors live on om row 0 (cols 14:142 b1, 142:206 b2);
        # ones row at partition 0 for psum bias preloads
        onesrow = work.tile([1, WIN * 25], BF16, tag="onesrow")
        nc.gpsimd.memset(onesrow[:], 1.0)

        # ---------- encoder matmuls: feats [d, tau] in two d-halves ----
        feats0 = ps.tile([128, WIN], F32, tag="feats0")
        feats1 = ps.tile([128, WIN], F32, tag="feats1")
        fps = [feats0, feats1]
        for g in range(4):
            wk = wkp.tile([128, 24 * D], F8, tag="wk")
            nc.sync.dma_start(wk[:], a_in["wencT"][g])
            for r in range(24):
                ki = g * 24 + r
                for u in range(2):
                    nc.tensor.matmul(
                        fps[u][:],
                        wk[:, r * D + u * 128: r * D + (u + 1) * 128],
                        xt[:, ki * WIN: (ki + 1) * WIN],
                        start=(ki == 0),
                        stop=(ki == 95),
                    )

        # conv weights after wencT (conv chain runs later anyway); wc1 in
        # two pieces so conv1 phase A starts on the first
        wc1 = io.tile([128, 54 * 128 + 576], F8)
        wc2kh0 = wc1[:, 54 * 128: 54 * 128 + 576]
        wc23t = io.tile([128, 1476], mybir.dt.uint8)
        wc2r = wc23t[:, 0:1152].bitcast(F8)
        wc3 = wc23t[:, 1152:1476].bitcast(BF16)
        nc.sync.dma_start(wc1[:, 0:36 * 128], a_in["wc1"][:, 0:36 * 128])
        nc.sync.dma_start(wc1[:, 36 * 128:], a_in["wc1"][:, 36 * 128:])
        nc.sync.dma_start(wc23t[:], a_in["wc23"])

        # feats + b_enc (per-partition broadcast add) -> fpad on DVE
        for u in range(2):
            nc.vector.tensor_add(
                fpad[:, u * WP + 1: u * WP + 1 + WIN], fps[u][:],
                om[:, 206 + u: 207 + u].broadcast_to([128, WIN]),
            )

        # ---------- mask stats (DVE; overlapped with DMA/encoder) ------
        mo = work.tile([128, 1056], BF16, tag="mo")
        for c in range(C):
            sl = slice(c * 32 * WIN, (c + 1) * 32 * WIN)
            nc.vector.tensor_mul(mo[:, sl], obs[:, sl], mst)
        mo2 = work.tile([128, 1056], BF16, tag="mo2")
        nc.vector.tensor_mul(mo2[:], mo[:], obs)
        smv = work.tile([128, NSTAT], BF16, tag="smv")
        vmo = mo[:].rearrange("p (c h t) -> p c t h", c=3, h=32, t=WIN)
        vms = mst.rearrange("p (h t) -> p t h", h=32, t=WIN)
        vU1 = smv[:, 0:NU1].rearrange("p (c g t) -> p c g t", c=3, g=NG, t=WIN)
        vUc = smv[:, NU1:NU1 + NUC].rearrange("p (g t) -> p g t", g=NG, t=WIN)
        with nc.allow_low_precision(reason="short class sums; bf16 ok"):
            for gi, (h0, h1_) in enumerate(H2G):
                nc.vector.reduce_sum(
                    vU1[:, :, gi, :], vmo[:, :, :, h0:h1_],
                    axis=mybir.AxisListType.X,
                )
                nc.vector.reduce_sum(
                    vUc[:, gi, :], vms[:, :, h0:h1_], axis=mybir.AxisListType.X
                )
            nc.vector.reduce_sum(
                smv[:, NSTAT - 1: NSTAT], mo2[:], axis=mybir.AxisListType.X
            )
        # class matmul: [14, NSTAT] = wclsT^T @ smv   (after encoder on PE)
        sps = ps.tile([14, NSTAT], F32, tag="stat")
        nc.tensor.matmul(sps[:], om[:, 0:14], smv[:], start=True, stop=True)
        outv = work.tile([14, NSTAT], F32, tag="outv")
        nc.scalar.activation(
            outv[:], sps[:], mybir.ActivationFunctionType.Identity
        )
        nc.sync.dma_start(a_out["outv"], outv[:])

        # ---------- conv1: direct 5x5 grid via W1eff variants ----------
        # phase A groups use variants in the first wc1 piece (v <= 4)
        groups = sorted(
            ((a5, b5) for a5 in range(5) for b5 in range(5)),
            key=lambda ab: (M35[ab[0]] * 3 + M35[ab[1]] > 5, M35[ab[0]] * 3 + M35[ab[1]] > 2),
        )
        c1a = ps.tile([128, WIN * 4 * 5], F32, tag="c1a")
        c1b = ps.tile([128, WIN * 1 * 5], F32, tag="c1b")
        vc1a = c1a[:].rearrange("p (t a b) -> p t a b", t=WIN, a=4, b=5)
        vc1b = c1b[:].rearrange("p (t a b) -> p t a b", t=WIN, a=1, b=5)
        vh1p = h1p[:].rearrange("p (t a b) -> p t a b", t=WP, a=7, b=7)
        c2 = ps.tile([64, WIN * 5 * 5], F32, tag="c2")

        def c1_group(a5, b5):
            v = M35[a5] * 3 + M35[b5]
            dst = vc1a[:, :, a5, b5] if a5 < 4 else vc1b[:, :, 0, b5]
            nc.tensor.matmul(
                dst, om[0:1, 14:142], onesrow[:, 0:WIN],
                start=True, stop=False,
            )
            for kt in range(3):
                for u in range(2):
                    nc.tensor.matmul(
                        dst,
                        wc1[:, ((v * 3 + kt) * 2 + u) * 128:
                            ((v * 3 + kt) * 2 + u + 1) * 128],
                        fpad[:, u * WP + kt: u * WP + kt + WIN],
                        start=False,
                        stop=(kt == 2 and u == 1),
                    )

        def c2_taps(kh):
            for kt in range(3):
                for kw in range(3):
                    if kh == 0:
                        wsl = wc2kh0[:, (kt * 3 + kw) * 64:
                                     (kt * 3 + kw + 1) * 64]
                    else:
                        t_r = ((kh - 1) * 3 + kt) * 3 + kw
                        wsl = wc2r[:, t_r * 64: (t_r + 1) * 64]
                    nc.tensor.matmul(
                        c2[:],
                        wsl,
                        vh1p[:, kt:kt + WIN, kh:kh + 5, kw:kw + 5],
                        start=False,
                        stop=(kh == 2 and kt == 2 and kw == 2),
                    )

        # conv2 accumulation opens with the bias preload (no data deps)
        nc.tensor.matmul(
            c2[:], om[0:1, 142:206], onesrow[:, 0:WIN * 25],
            start=True, stop=False,
        )
        # conv1 phase A (a5 0..3; weights in wc1 pieces 1+2)
        for a5, b5 in groups[:20]:
            c1_group(a5, b5)
        nc.vector.tensor_relu(
            vh1p[:, 1:1 + WIN, 1:5, 1:6], vc1a[:]
        )
        # conv1 phase B (a5 = 4; weights in wc1 piece 2)
        for a5, b5 in groups[20:]:
            c1_group(a5, b5)
        nc.vector.tensor_relu(
            vh1p[:, 1:1 + WIN, 5:6, 1:6], vc1b[:]
        )
        c2_taps(0)
        c2_taps(1)
        c2_taps(2)
        # relu -> h2 (bf16) on DVE, then a-axis-only 5->7 expansion
        # (conv3 addresses the b-axis 5-grid directly via B5MAP)
        h2 = work.tile([64, WIN * 5 * 5], BF16, tag="h2")
        nc.vector.tensor_relu(h2[:], c2[:])
        # src dims permuted to (b5, t, a5) to match h2p5 layout
        vh2b = h2[:].rearrange("p (t a b) -> p b t a", t=WIN, a=5, b=5)
        vh2p = h2p[0:64, 0:7 * WP * 9].rearrange(
            "p (b t a) -> p b t a", b=7, t=WP, a=9)
        for ci, (da, la, sa, lsa) in enumerate(G57):
            src = vh2b[:, :, :, sa:sa + lsa]
            if lsa == 1:
                src = src.broadcast_to([64, 5, WIN, la])
            dst = vh2p[:, 1:6, 1:1 + WIN, 1 + da:1 + da + la]
            nc.vector.tensor_copy(dst, src)

        # ---------- conv3: vox-stationary (flat 99-col slices; 2 junk
        # a-rows per tau that the host ignores), b3 folded via ones row.
        # kw-taps hitting the same 5-grid b-row merge (the flat-run base
        # is kw-independent, so presummed weights are exact); taps whose
        # b-row is pure pad contribute zero and are skipped entirely.
        NVX = 9 * WIN  # 99
        B5MAP = [0, 1, 2, 3, 3, 3, 4, 5, 6]
        VT = {(0, 1): 0, (1, 2): 1, (0, 1, 2): 2}  # kw-set -> variant idx
        c3 = ps.tile([NVX, 21], F32, tag="c3")
        for bc in range(7):
            # group kws by target b-row, dropping pad rows 0 and 6
            bygrp = {}
            for kw in range(3):
                row = B5MAP[bc + kw]
                if row in (0, 6):
                    continue
                bygrp.setdefault(row, []).append(kw)
            grps = sorted(bygrp.items())
            nmm = 9 * len(grps)
            mi = 0
            for kt in range(3):
                for kh in range(3):
                    for row, kws in grps:
                        if len(kws) == 1:
                            col = ((kt * 3 + kh) * 3 + kws[0]) * 3
                        else:
                            vt = VT[tuple(kws)]
                            col = 81 + (vt * 9 + kt * 3 + kh) * 3
                        rows = 65 if mi == 0 else 64
                        base = row * WP * 9 + kt * 9 + kh
                        nc.tensor.matmul(
                            c3[:, bc * 3: (bc + 1) * 3],
                            h2p[0:rows, base: base + NVX],
                            wc3[0:rows, col: col + 3],
                            start=(mi == 0),
                            stop=(mi == nmm - 1),
                        )
                        mi += 1
        recon = work.tile([NVX, 21], F32, tag="recon")
        nc.vector.tensor_copy(recon[:], c3[:])
        nc.sync.dma_start(a_out["recon"], recon[:])


_CACHE = {}


def _build():
    if "nc" in _CACHE:
        return _CACHE["nc"]
    nc = bacc.Bacc("TRN2", target_bir_lowering=False, debug=False)
    a_in = {}

    def din(name, shape, dt):
        a_in[name] = nc.dram_tensor(name, shape, dt, kind="ExternalInput").ap()

    din("in0", (128, 2176), mybir.dt.uint8)
    din("wencT", (4, 128, 24 * D), F8)
    din("wc1", (128, 54 * 128 + 576), F8)
    din("wc23", (128, 1476), mybir.dt.uint8)
    a_out = {}
    for name, shape in [("recon", (9 * WIN, 21)), ("outv", (14, NSTAT))]:
        a_out[name] = nc.dram_tensor(name, shape, F32, kind="ExternalOutput").ap()
    _emit(nc, a_in, a_out)
    nc.compile()
    _CACHE["nc"] = nc
    return nc


def make_in_maps(obs_strip, mask, W_enc, b_enc, w1, b1, w2, b2, w3, b3):
    import ml_dtypes

    bf16 = ml_dtypes.bfloat16
    f8 = ml_dtypes.float8_e4m3

    obs_strip = np.asarray(obs_strip, np.float32)
    mask_f = np.asarray(mask).astype(np.float32)

    # --- shared weights ---
    wencT = np.ascontiguousarray(
        np.asarray(W_enc, np.float32)
        .reshape(D, 3, 32, 2, 64)
        .transpose(3, 4, 1, 2, 0)
        .reshape(128, 96, D)
        .reshape(128, 4, 24 * D)
        .transpose(1, 0, 2)
    ).astype(f8)

    K = {0: [1, 2], 1: [0, 1, 2], 2: [0, 1]}
    w1 = np.asarray(w1, np.float32)
    W1e = np.zeros((9, 3, 128, 2, 128), np.float32)  # [v, kt, c, u, dmod]
    for va in range(3):
        for vb in range(3):
            for kt in range(3):
                eff = w1[:, :, kt][:, :, K[va]][:, :, :, K[vb]].sum((2, 3))
                W1e[va * 3 + vb, kt] = eff.reshape(128, 2, 128)
    wc1 = np.zeros((128, 54 * 128 + 576), ml_dtypes.float8_e4m3)
    wc1[:, 0:54 * 128] = np.ascontiguousarray(
        W1e.transpose(4, 0, 1, 3, 2).reshape(128, 54 * 128)
    ).astype(f8)

    # [cin, kh, kt, kw, cout]; kh=0 block rides in wc1's tail
    wc2 = np.ascontiguousarray(
        np.asarray(w2, np.float32).transpose(1, 3, 2, 4, 0).reshape(128, 27 * 64)
    ).astype(f8)
    wc1[:, 54 * 128:] = wc2[:, 0:576]

    wc3 = np.zeros((128, 162), np.float32)
    w3v = np.asarray(w3, np.float32).transpose(1, 2, 3, 4, 0)  # [cin,kt,kh,kw,co]
    wc3[0:64, 0:81] = w3v.reshape(64, 81)
    for vt, kws in [(0, (0, 1)), (1, (1, 2)), (2, (0, 1, 2))]:
        s = w3v[:, :, :, list(kws), :].sum(3)  # [cin, kt, kh, co]
        wc3[0:64, 81 + vt * 27: 81 + (vt + 1) * 27] = s.reshape(64, 27)
    b3f = np.asarray(b3, np.float32)
    for c0 in (0, 3, 81, 135):  # first-tap cols of bc 1/2/5/6, 0, 4, 3
        wc3[64, c0: c0 + 3] = b3f
    wc3 = wc3.astype(bf16)
    wc23 = np.zeros((128, 1476), np.uint8)
    wc23[:, 0:1152] = wc2[:, 576:1728].view(np.uint8)
    wc23[:, 1152:1476] = wc3.view(np.uint8)
    wcls = np.zeros((128, 14), np.float32)
    for u in range(2):
        for j in range(7):
            w0, w1_ = WCLS_BOUNDS[j], WCLS_BOUNDS[j + 1]
            wcls[u * 64 + w0: u * 64 + w1_, u * 7 + j] = 1.0

    om = np.zeros((128, 208), np.float32)
    om[:, 0:14] = wcls
    om[0, 14:142] = np.asarray(b1, np.float32)
    om[0, 142:206] = np.asarray(b2, np.float32)
    om[:, 206] = np.asarray(b_enc, np.float32)[0:128]
    om[:, 207] = np.asarray(b_enc, np.float32)[128:256]
    om_u8 = np.ascontiguousarray(om.astype(bf16)).view(np.uint8)

    shared = {"wencT": wencT, "wc1": wc1, "wc23": wc23}

    def perm_obs(o):  # [t, C, H, W] -> [128, (c, h2, t)]
        t = o.shape[0]
        return (o.reshape(t, 3, 32, 2, 64).transpose(3, 4, 1, 2, 0)
                .reshape(128, 3 * 32 * t))

    def perm_msk(m):  # [t, H, W] -> [128, (h2, t)]
        t = m.shape[0]
        return (m.reshape(t, 32, 2, 64).transpose(2, 3, 1, 0)
                .reshape(128, 32 * t))

    in_maps = []
    for core in range(NCORES):
        b, th = core // 2, core % 2
        s = 5 * th
        in0 = np.zeros((128, 2176), np.uint8)
        in0[:, 0:416] = om_u8
        in0[:, 416:1472] = perm_obs(obs_strip[b, s:s + WIN]).astype(f8).view(np.uint8)
        in0[:, 1472:1824] = perm_msk(1.0 - mask_f[b, s:s + WIN])
        mstat = mask_f[b].copy()
        if th == 0:
            mstat[8:] = 0.0
        else:
            mstat[:8] = 0.0
        in0[:, 1824:2176] = perm_msk(mstat[s:s + WIN])
        in_maps.append({"in0": in0, **shared})
    return in_maps


# host-side fold: (g, hpar) -> h class contributions
HCLS_SRC = [[(0, 0)], [(0, 1)], [(1, 0)],
            [(1, 1), (2, 0), (2, 1), (3, 0)],
            [(3, 1)], [(4, 0)], [(4, 1)]]


def assemble(results):
    total_sq = 0.0
    total_cnt = 0.0
    total_s2 = 0.0
    for core in range(NCORES):
        r = results[core]
        rec = r["recon"].astype(np.float64).reshape(WIN, 9, 7, 3)[:, 0:7]  # [tau,a,b,c]
        outv = r["outv"].astype(np.float64)
        U1 = outv[:, 0:NU1].reshape(2, 7, 3, NG, WIN)   # [u,j,c,g,tau]
        Uc = outv[:, NU1:NU1 + NUC].reshape(2, 7, NG, WIN)  # [u,j,g,tau]
        total_s2 += float(outv[:, NSTAT - 1].sum())
        s1 = np.zeros((3, 7, 7, WIN))   # [c, hcls, wcls, tau]
        cnt = np.zeros((7, 7, WIN))     # [hcls, wcls, tau]
        for i in range(7):
            for (g, u) in HCLS_SRC[i]:
                s1[:, i] += U1[u, :, :, g, :].transpose(1, 0, 2)
                cnt[i] += Uc[u, :, g, :]
        rt = rec.transpose(3, 1, 2, 0)  # [c, a(hcls), b(wcls), tau]
        total_sq += float((rt * rt * cnt[None]).sum() - 2.0 * (rt * s1).sum())
        total_cnt += float(cnt.sum())
    loss = (total_sq + total_s2) / max(total_cnt * C, 1.0)
    return np.float32(loss)


def kernel(**inputs):
    nc = _build()
    in_maps = make_in_maps(**inputs)
    res = bass_utils.run_bass_kernel_spmd(nc, in_maps, core_ids=list(range(NCORES)))
    _CACHE["last_res"] = res
    return assemble(res.results)


if __name__ == "__main__":
    pass


# revision 81
# speedup vs baseline: 1.0040x; 1.0040x over previous
"""Masked video loss kernel for TRN2 (8 NeuronCores, SPMD).

Algorithmic structure exploited:
- The decoder input feat_3d is spatially constant (broadcast of per-frame
  features over H=W=64), so conv1 collapses to a per-frame linear map with
  9 edge-variant weight sums (W1eff), evaluated directly on a 5x5 class
  grid. conv2 runs as a true 3x3x3 conv on the (padded) 5-grid; its output
  is expanded to the 7-grid along the h-axis only (conv3's stationary
  slices address the w-axis 5-grid directly via a b-row lookup) and conv3
  produces the 7x7 recon classes. All exact (class algebra), not
  approximations.
- Masked MSE folds through per-class stats: sum (r-o)^2 = r^2 cnt - 2 r s1
  + s2 per (t, 7x7 class); s1/cnt come from one 0/1-matrix PE matmul plus
  segmented DVE reduces.

Sharding: core = 2*b + th. Each core handles batch b and an 11-frame
t-window starting at s = 5*th (host shifts the data, so the program is
SPMD-uniform); decoder outputs are valid for the core's 8-frame t-half,
and mask stats are host-zeroed outside that half.

Precision: W_enc / w1eff / w2 / obs in fp8-e4m3, masks u8, activations
bf16, accumulation fp32. Measured end-to-end loss rel err 3.2e-3
(harness gate 2e-2).
"""

import sys

sys.path.insert(0, "/opt/trn_rl_repo")

from contextlib import ExitStack  # noqa: E402

import numpy as np  # noqa: E402

import concourse.bacc as bacc  # noqa: E402
import concourse.mybir as mybir  # noqa: E402
import concourse.tile as tile  # noqa: E402
from concourse import bass_utils  # noqa: E402

B, T, C, H, W = 4, 16, 3, 64, 64
D = 256
NCORES = 8

F32 = mybir.dt.float32
BF16 = mybir.dt.bfloat16
F8 = mybir.dt.float8e4

WIN = 11          # feats/conv t-window frames per core
WP = WIN + 2      # padded window
M35 = [0, 1, 1, 1, 2]          # 5-grid pos -> 3-class variant
M57 = [0, 1, 2, 2, 2, 3, 4]    # 7-grid pos -> 5-grid src index
# expansion groups (dst0, dstlen, src0, srclen) along one axis for 5->7
G57 = [(0, 2, 0, 2), (2, 3, 2, 1), (5, 2, 3, 2)]
# h2-row groups for segmented stats reduction (h = 2*h2 + hpar)
H2G = [(0, 1), (1, 2), (2, 30), (30, 31), (31, 32)]
NG = len(H2G)
WCLS_BOUNDS = [0, 1, 2, 3, 61, 62, 63, 64]

NU1 = 3 * NG * WIN       # 165
NUC = NG * WIN           # 55
NSTAT = NU1 + NUC + 1    # 221


def _emit(nc, a_in, a_out):
    ctx = ExitStack()
    tc = tile.TileContext(nc)
    with tc, ctx:
        io = ctx.enter_context(tc.tile_pool(name="io", bufs=1))
        wkp = ctx.enter_context(tc.tile_pool(name="wkp", bufs=3))
        work = ctx.enter_context(tc.tile_pool(name="work", bufs=1))
        ps = ctx.enter_context(tc.tile_pool(name="ps", bufs=1, space="PSUM"))

        # ---------- early memsets (Pool; no deps) ----------
        fpad = work.tile([128, 2 * WP], BF16, tag="fpad")
        nc.gpsimd.memset(fpad[:], 0.0)
        h1p = work.tile([128, WP * 7 * 7], BF16, tag="h1p")
        nc.gpsimd.memset(h1p[:], 0.0)
        h2p = work.tile([65, 7 * WP * 9 + 2], BF16, tag="h2p")
        nc.gpsimd.memset(h2p[0:64, :], 0.0)
        nc.gpsimd.memset(h2p[64:65, :], 1.0)

        # ---------- input DMAs (serialized by the DMA engine) ----------
        # one merged leading tensor: [bf16 consts block | fp8 obs | u8 masks]
        in0 = io.tile([128, 2176], mybir.dt.uint8)
        nc.sync.dma_start(in0[:], a_in["in0"])
        om = in0[:, 0:416].bitcast(BF16)   # [128, 208]
        obs8 = in0[:, 416:1472].bitcast(F8)
        msk8 = in0[:, 1472:2176]
        obsb = work.tile([128, 1056], BF16, tag="obsb")
        obs = obsb[:, 0:1056]
        kmbf = work.tile([128, 704], BF16, tag="kmbf")
        keep = kmbf[:, 0:352]
        mst = kmbf[:, 352:704]
        nc.vector.tensor_copy(keep, msk8[:, 0:352])  # u8 -> bf16

        # ---------- PE warm-up (p-state ramp) ----------
        warm_ps = ps.tile([2, 512], F32, tag="warm")
        for i in range(24):
            nc.tensor.matmul(
                warm_ps[:, 0:8], om[:, 0:2], om[:, 0:8],
                start=(i == 0), stop=(i == 23),
            )
        # early act-table preload (off the critical path)
        junk = work.tile([2, 8], F32, tag="junk")
        nc.scalar.activation(
            junk[:], warm_ps[:, 0:8], mybir.ActivationFunctionType.Relu
        )
        for i in range(10):
            nc.tensor.matmul(
                warm_ps[:, 0:128], om[:, 0:2], om[:, 0:128],
                start=(i == 0), stop=(i == 9),
            )

        # ---------- encoder input masking (cast + mask per c-block) ----
        xt = work.tile([128, 3 * 32 * WIN], BF16, tag="xt")
        for c in range(C):
            sl = slice(c * 32 * WIN, (c + 1) * 32 * WIN)
            nc.vector.tensor_copy(obsb[:, sl], obs8[:, sl])
            nc.vector.tensor_mul(xt[:, sl], obs[:, sl], keep)
        nc.vector.tensor_copy(mst, msk8[:, 352:704])

        # bias vectors live on om row 0 (cols 14:142 b1, 142:206 b2);
        # ones row at partition 0 for psum bias preloads
        onesrow = work.tile([1, WIN * 25], BF16, tag="onesrow")
        nc.gpsimd.memset(onesrow[:], 1.0)

        # ---------- encoder matmuls: feats [d, tau] in two d-halves ----
        feats0 = ps.tile([128, WIN], F32, tag="feats0")
        feats1 = ps.tile([128, WIN], F32, tag="feats1")
        fps = [feats0, feats1]
        for g in range(4):
            wk = wkp.tile([128, 24 * D], F8, tag="wk")
            nc.sync.dma_start(wk[:], a_in["wencT"][g])
            for r in range(24):
                ki = g * 24 + r
                for u in range(2):
                    nc.tensor.matmul(
                        fps[u][:],
                        wk[:, r * D + u * 128: r * D + (u + 1) * 128],
                        xt[:, ki * WIN: (ki + 1) * WIN],
                        start=(ki == 0),
                        stop=(ki == 95),
                    )

        # conv weights after wencT (conv chain runs later anyway); wc1 in
        # two pieces so conv1 phase A starts on the first
        wc1 = io.tile([128, 54 * 128 + 576], F8)
        wc2kh0 = wc1[:, 54 * 128: 54 * 128 + 576]
        wc23t = io.tile([128, 1476], mybir.dt.uint8)
        wc2r = wc23t[:, 0:1152].bitcast(F8)
        wc3 = wc23t[:, 1152:1476].bitcast(BF16)
        nc.sync.dma_start(wc1[:, 0:36 * 128], a_in["wc1"][:, 0:36 * 128])
        nc.sync.dma_start(wc1[:, 36 * 128:], a_in["wc1"][:, 36 * 128:])
        nc.sync.dma_start(wc23t[:], a_in["wc23"])

        # feats + b_enc (per-partition broadcast add) -> fpad on DVE
        for u in range(2):
            nc.vector.tensor_add(
                fpad[:, u * WP + 1: u * WP + 1 + WIN], fps[u][:],
                om[:, 206 + u: 207 + u].broadcast_to([128, WIN]),
            )

        # ---------- mask stats (DVE; overlapped with DMA/encoder) ------
        mo = work.tile([128, 1056], BF16, tag="mo")
        for c in range(C):
            sl = slice(c * 32 * WIN, (c + 1) * 32 * WIN)
            nc.vector.tensor_mul(mo[:, sl], obs[:, sl], mst)
        mo2 = work.tile([128, 1056], BF16, tag="mo2")
        nc.vector.tensor_mul(mo2[:], mo[:], obs)
        smv = work.tile([128, NSTAT], BF16, tag="smv")
        vmo = mo[:].rearrange("p (c h t) -> p c t h", c=3, h=32, t=WIN)
        vms = mst.rearrange("p (h t) -> p t h", h=32, t=WIN)
        vU1 = smv[:, 0:NU1].rearrange("p (c g t) -> p c g t", c=3, g=NG, t=WIN)
        vUc = smv[:, NU1:NU1 + NUC].rearrange("p (g t) -> p g t", g=NG, t=WIN)
        with nc.allow_low_precision(reason="short class sums; bf16 ok"):
            for gi, (h0, h1_) in enumerate(H2G):
                nc.vector.reduce_sum(
                    vU1[:, :, gi, :], vmo[:, :, :, h0:h1_],
                    axis=mybir.AxisListType.X,
                )
                nc.vector.reduce_sum(
                    vUc[:, gi, :], vms[:, :, h0:h1_], axis=mybir.AxisListType.X
                )
            nc.vector.reduce_sum(
                smv[:, NSTAT - 1: NSTAT], mo2[:], axis=mybir.AxisListType.X
            )
        # class matmul: [14, NSTAT] = wclsT^T @ smv   (after encoder on PE)
        sps = ps.tile([14, NSTAT], F32, tag="stat")
        nc.tensor.matmul(sps[:], om[:, 0:14], smv[:], start=True, stop=True)
        outv = work.tile([14, NSTAT], F32, tag="outv")
        nc.scalar.activation(
            outv[:], sps[:], mybir.ActivationFunctionType.Identity
        )
        nc.sync.dma_start(a_out["outv"], outv[:])

        # ---------- conv1: direct 5x5 grid via W1eff variants ----------
        # phase A groups use variants in the first wc1 piece (v <= 4)
        groups = sorted(
            ((a5, b5) for a5 in range(5) for b5 in range(5)),
            key=lambda ab: (M35[ab[0]] * 3 + M35[ab[1]] > 5, M35[ab[0]] * 3 + M35[ab[1]] > 2),
        )
        c1a = ps.tile([128, WIN * 4 * 5], F32, tag="c1a")
        c1b = ps.tile([128, WIN * 1 * 5], F32, tag="c1b")
        vc1a = c1a[:].rearrange("p (t a b) -> p t a b", t=WIN, a=4, b=5)
        vc1b = c1b[:].rearrange("p (t a b) -> p t a b", t=WIN, a=1, b=5)
        vh1p = h1p[:].rearrange("p (t a b) -> p t a b", t=WP, a=7, b=7)
        c2 = ps.tile([64, WIN * 5 * 5], F32, tag="c2")

        def c1_dst(a5, b5):
            return vc1a[:, :, a5, b5] if a5 < 4 else vc1b[:, :, 0, b5]

        # bias preloads for all 25 groups run in the pre-wc1a idle (no
        # weight dependency), keeping them off the wc1a-gated stream
        for a5, b5 in groups:
            nc.tensor.matmul(
                c1_dst(a5, b5), om[0:1, 14:142], onesrow[:, 0:WIN],
                start=True, stop=False,
            )

        def c1_group(a5, b5):
            v = M35[a5] * 3 + M35[b5]
            dst = c1_dst(a5, b5)
            for kt in range(3):
                for u in range(2):
                    nc.tensor.matmul(
                        dst,
                        wc1[:, ((v * 3 + kt) * 2 + u) * 128:
                            ((v * 3 + kt) * 2 + u + 1) * 128],
                        fpad[:, u * WP + kt: u * WP + kt + WIN],
                        start=False,
                        stop=(kt == 2 and u == 1),
                    )

        def c2_taps(kh):
            for kt in range(3):
                for kw in range(3):
                    if kh == 0:
                        wsl = wc2kh0[:, (kt * 3 + kw) * 64:
                                     (kt * 3 + kw + 1) * 64]
                    else:
                        t_r = ((kh - 1) * 3 + kt) * 3 + kw
                        wsl = wc2r[:, t_r * 64: (t_r + 1) * 64]
                    nc.tensor.matmul(
                        c2[:],
                        wsl,
                        vh1p[:, kt:kt + WIN, kh:kh + 5, kw:kw + 5],
                        start=False,
                        stop=(kh == 2 and kt == 2 and kw == 2),
                    )

        # conv2 accumulation opens with the bias preload (no data deps)
        nc.tensor.matmul(
            c2[:], om[0:1, 142:206], onesrow[:, 0:WIN * 25],
            start=True, stop=False,
        )
        # conv1 phase A (a5 0..3; weights in wc1 pieces 1+2)
        for a5, b5 in groups[:20]:
            c1_group(a5, b5)
        nc.vector.tensor_relu(
            vh1p[:, 1:1 + WIN, 1:5, 1:6], vc1a[:]
        )
        # conv1 phase B (a5 = 4; weights in wc1 piece 2)
        for a5, b5 in groups[20:]:
            c1_group(a5, b5)
        nc.vector.tensor_relu(
            vh1p[:, 1:1 + WIN, 5:6, 1:6], vc1b[:]
        )
        c2_taps(0)
        c2_taps(1)
        c2_taps(2)
        # relu -> h2 (bf16) on DVE, then a-axis-only 5->7 expansion
        # (conv3 addresses the b-axis 5-grid directly via B5MAP)
        h2 = work.tile([64, WIN * 5 * 5], BF16, tag="h2")
        nc.vector.tensor_relu(h2[:], c2[:])
        # src dims permuted to (b5, t, a5) to match h2p5 layout
        vh2b = h2[:].rearrange("p (t a b) -> p b t a", t=WIN, a=5, b=5)
        vh2p = h2p[0:64, 0:7 * WP * 9].rearrange(
            "p (b t a) -> p b t a", b=7, t=WP, a=9)
        for ci, (da, la, sa, lsa) in enumerate(G57):
            src = vh2b[:, :, :, sa:sa + lsa]
            if lsa == 1:
                src = src.broadcast_to([64, 5, WIN, la])
            dst = vh2p[:, 1:6, 1:1 + WIN, 1 + da:1 + da + la]
            nc.vector.tensor_copy(dst, src)

        # ---------- conv3: vox-stationary (flat 99-col slices; 2 junk
        # a-rows per tau that the host ignores), b3 folded via ones row.
        # kw-taps hitting the same 5-grid b-row merge (the flat-run base
        # is kw-independent, so presummed weights are exact); taps whose
        # b-row is pure pad contribute zero and are skipped entirely.
        NVX = 9 * WIN  # 99
        B5MAP = [0, 1, 2, 3, 3, 3, 4, 5, 6]
        VT = {(0, 1): 0, (1, 2): 1, (0, 1, 2): 2}  # kw-set -> variant idx
        c3 = ps.tile([NVX, 21], F32, tag="c3")
        for bc in range(7):
            # group kws by target b-row, dropping pad rows 0 and 6
            bygrp = {}
            for kw in range(3):
                row = B5MAP[bc + kw]
                if row in (0, 6):
                    continue
                bygrp.setdefault(row, []).append(kw)
            grps = sorted(bygrp.items())
            nmm = 9 * len(grps)
            mi = 0
            for kt in range(3):
                for kh in range(3):
                    for row, kws in grps:
                        if len(kws) == 1:
                            col = ((kt * 3 + kh) * 3 + kws[0]) * 3
                        else:
                            vt = VT[tuple(kws)]
                            col = 81 + (vt * 9 + kt * 3 + kh) * 3
                        rows = 65 if mi == 0 else 64
                        base = row * WP * 9 + kt * 9 + kh
                        nc.tensor.matmul(
                            c3[:, bc * 3: (bc + 1) * 3],
                            h2p[0:rows, base: base + NVX],
                            wc3[0:rows, col: col + 3],
                            start=(mi == 0),
                            stop=(mi == nmm - 1),
                        )
                        mi += 1
        recon = work.tile([NVX, 21], F32, tag="recon")
        nc.vector.tensor_copy(recon[:], c3[:])
        nc.sync.dma_start(a_out["recon"], recon[:])


_CACHE = {}


def _build():
    if "nc" in _CACHE:
        return _CACHE["nc"]
    nc = bacc.Bacc("TRN2", target_bir_lowering=False, debug=False)
    a_in = {}

    def din(name, shape, dt):
        a_in[name] = nc.dram_tensor(name, shape, dt, kind="ExternalInput").ap()

    din("in0", (128, 2176), mybir.dt.uint8)
    din("wencT", (4, 128, 24 * D), F8)
    din("wc1", (128, 54 * 128 + 576), F8)
    din("wc23", (128, 1476), mybir.dt.uint8)
    a_out = {}
    for name, shape in [("recon", (9 * WIN, 21)), ("outv", (14, NSTAT))]:
        a_out[name] = nc.dram_tensor(name, shape, F32, kind="ExternalOutput").ap()
    _emit(nc, a_in, a_out)
    nc.compile()
    _CACHE["nc"] = nc
    return nc


def make_in_maps(obs_strip, mask, W_enc, b_enc, w1, b1, w2, b2, w3, b3):
    import ml_dtypes

    bf16 = ml_dtypes.bfloat16
    f8 = ml_dtypes.float8_e4m3

    obs_strip = np.asarray(obs_strip, np.float32)
    mask_f = np.asarray(mask).astype(np.float32)

    # --- shared weights ---
    wencT = np.ascontiguousarray(
        np.asarray(W_enc, np.float32)
        .reshape(D, 3, 32, 2, 64)
        .transpose(3, 4, 1, 2, 0)
        .reshape(128, 96, D)
        .reshape(128, 4, 24 * D)
        .transpose(1, 0, 2)
    ).astype(f8)

    K = {0: [1, 2], 1: [0, 1, 2], 2: [0, 1]}
    w1 = np.asarray(w1, np.float32)
    W1e = np.zeros((9, 3, 128, 2, 128), np.float32)  # [v, kt, c, u, dmod]
    for va in range(3):
        for vb in range(3):
            for kt in range(3):
                eff = w1[:, :, kt][:, :, K[va]][:, :, :, K[vb]].sum((2, 3))
                W1e[va * 3 + vb, kt] = eff.reshape(128, 2, 128)
    wc1 = np.zeros((128, 54 * 128 + 576), ml_dtypes.float8_e4m3)
    wc1[:, 0:54 * 128] = np.ascontiguousarray(
        W1e.transpose(4, 0, 1, 3, 2).reshape(128, 54 * 128)
    ).astype(f8)

    # [cin, kh, kt, kw, cout]; kh=0 block rides in wc1's tail
    wc2 = np.ascontiguousarray(
        np.asarray(w2, np.float32).transpose(1, 3, 2, 4, 0).reshape(128, 27 * 64)
    ).astype(f8)
    wc1[:, 54 * 128:] = wc2[:, 0:576]

    wc3 = np.zeros((128, 162), np.float32)
    w3v = np.asarray(w3, np.float32).transpose(1, 2, 3, 4, 0)  # [cin,kt,kh,kw,co]
    wc3[0:64, 0:81] = w3v.reshape(64, 81)
    for vt, kws in [(0, (0, 1)), (1, (1, 2)), (2, (0, 1, 2))]:
        s = w3v[:, :, :, list(kws), :].sum(3)  # [cin, kt, kh, co]
        wc3[0:64, 81 + vt * 27: 81 + (vt + 1) * 27] = s.reshape(64, 27)
    b3f = np.asarray(b3, np.float32)
    for c0 in (0, 3, 81, 135):  # first-tap cols of bc 1/2/5/6, 0, 4, 3
        wc3[64, c0: c0 + 3] = b3f
    wc3 = wc3.astype(bf16)
    wc23 = np.zeros((128, 1476), np.uint8)
    wc23[:, 0:1152] = wc2[:, 576:1728].view(np.uint8)
    wc23[:, 1152:1476] = wc3.view(np.uint8)
    wcls = np.zeros((128, 14), np.float32)
    for u in range(2):
        for j in range(7):
            w0, w1_ = WCLS_BOUNDS[j], WCLS_BOUNDS[j + 1]
            wcls[u * 64 + w0: u * 64 + w1_, u * 7 + j] = 1.0

    om = np.zeros((128, 208), np.float32)
    om[:, 0:14] = wcls
    om[0, 14:142] = np.asarray(b1, np.float32)
    om[0, 142:206] = np.asarray(b2, np.float32)
    om[:, 206] = np.asarray(b_enc, np.float32)[0:128]
    om[:, 207] = np.asarray(b_enc, np.float32)[128:256]
    om_u8 = np.ascontiguousarray(om.astype(bf16)).view(np.uint8)

    shared = {"wencT": wencT, "wc1": wc1, "wc23": wc23}

    def perm_obs(o):  # [t, C, H, W] -> [128, (c, h2, t)]
        t = o.shape[0]
        return (o.reshape(t, 3, 32, 2, 64).transpose(3, 4, 1, 2, 0)
                .reshape(128, 3 * 32 * t))

    def perm_msk(m):  # [t, H, W] -> [128, (h2, t)]
        t = m.shape[0]
        return (m.reshape(t, 32, 2, 64).transpose(2, 3, 1, 0)
                .reshape(128, 32 * t))

    in_maps = []
    for core in range(NCORES):
        b, th = core // 2, core % 2
        s = 5 * th
        in0 = np.zeros((128, 2176), np.uint8)
        in0[:, 0:416] = om_u8
        in0[:, 416:1472] = perm_obs(obs_strip[b, s:s + WIN]).astype(f8).view(np.uint8)
        in0[:, 1472:1824] = perm_msk(1.0 - mask_f[b, s:s + WIN])
        mstat = mask_f[b].copy()
        if th == 0:
            mstat[8:] = 0.0
        else:
            mstat[:8] = 0.0
        in0[:, 1824:2176] = perm_msk(mstat[s:s + WIN])
        in_maps.append({"in0": in0, **shared})
    return in_maps


# host-side fold: (g, hpar) -> h class contributions
HCLS_SRC = [[(0, 0)], [(0, 1)], [(1, 0)],
            [(1, 1), (2, 0), (2, 1), (3, 0)],
            [(3, 1)], [(4, 0)], [(4, 1)]]


def assemble(results):
    total_sq = 0.0
    total_cnt = 0.0
    total_s2 = 0.0
    for core in range(NCORES):
        r = results[core]
        rec = r["recon"].astype(np.float64).reshape(WIN, 9, 7, 3)[:, 0:7]  # [tau,a,b,c]
        outv = r["outv"].astype(np.float64)
        U1 = outv[:, 0:NU1].reshape(2, 7, 3, NG, WIN)   # [u,j,c,g,tau]
        Uc = outv[:, NU1:NU1 + NUC].reshape(2, 7, NG, WIN)  # [u,j,g,tau]
        total_s2 += float(outv[:, NSTAT - 1].sum())
        s1 = np.zeros((3, 7, 7, WIN))   # [c, hcls, wcls, tau]
        cnt = np.zeros((7, 7, WIN))     # [hcls, wcls, tau]
        for i in range(7):
            for (g, u) in HCLS_SRC[i]:
                s1[:, i] += U1[u, :, :, g, :].transpose(1, 0, 2)
                cnt[i] += Uc[u, :, g, :]
        rt = rec.transpose(3, 1, 2, 0)  # [c, a(hcls), b(wcls), tau]
        total_sq += float((rt * rt * cnt[None]).sum() - 2.0 * (rt * s1).sum())
        total_cnt += float(cnt.sum())
    loss = (total_sq + total_s2) / max(total_cnt * C, 1.0)
    return np.float32(loss)


def kernel(**inputs):
    nc = _build()
    in_maps = make_in_maps(**inputs)
    res = bass_utils.run_bass_kernel_spmd(nc, in_maps, core_ids=list(range(NCORES)))
    _CACHE["last_res"] = res
    return assemble(res.results)


if __name__ == "__main__":
    pass
